# revision 11
# baseline (speedup 1.0000x reference)
"""Trainium2 Bass kernel for the sparse Lie-bracket bilinear layer.

  out[b, k] = alpha * sum_{t : idx_k[t]==k} coeff[t] * x[b, idx_i[t]] * y[b, idx_j[t]]

Strategy (data-parallel over batch across 8 NeuronCores, no collectives):
  - Host: sort structure-constant triples into 8 buckets by
    (i_half, j_half, k_half), pad each bucket to a multiple of 128 with
    zero-coeff triples -> chunks of 128 triples.
  - For each chunk build one-hot matrices (fp16, exact):
      Gi [128 i_local, 128 t]  so  Gi.T @ xT_half = x[idx_i, :]
      Gj [128 j_local, 128 t]
      S  [128 t, 128 k_local]  (pure 0/1; coeff applied separately in fp32)
  - xT / yT are split into fp16 hi/lo pairs on the host; the two gather
    matmuls accumulate in PSUM, reconstructing exact fp32 gathered values.
  - DVE computes vals = (xi * c) * yj in one fused scalar_tensor_tensor op
    (c = per-partition fp32 coeff scalars). vals is split into fp16 hi/lo
    and scatter-accumulated into persistent PSUM via two more matmuls.
"""

import numpy as np

import concourse.bass as bass
import concourse.mybir as mybir
from concourse import bacc
from concourse.tile import TileContext
from concourse.bass_utils import run_bass_kernel_spmd

NCORES = 8
P = 128

_PROG_CACHE = {}

LAST_RESULTS = None  # stash for test.py (exec time / profile)


def _build_program(n_chunks, meta, b_core, bt, n_bt):
    """meta: list of (ih, jh, kh) per chunk."""
    nc = bacc.Bacc("TRN2", target_bir_lowering=False, debug=False,
                   num_devices=NCORES)
    f16 = mybir.dt.float16
    f32 = mybir.dt.float32

    xt_hi = nc.dram_tensor("xt_hi", [2 * P, b_core], f16, kind="ExternalInput")
    xt_lo = nc.dram_tensor("xt_lo", [2 * P, b_core], f16, kind="ExternalInput")
    yt_hi = nc.dram_tensor("yt_hi", [2 * P, b_core], f16, kind="ExternalInput")
    yt_lo = nc.dram_tensor("yt_lo", [2 * P, b_core], f16, kind="ExternalInput")
    w = nc.dram_tensor("w", [n_chunks * P, 3 * P], f16, kind="ExternalInput")
    cv = nc.dram_tensor("cv", [P, n_chunks], f32, kind="ExternalInput")
    out = nc.dram_tensor("out", [2 * P, b_core], f32, kind="ExternalOutput")

    # last chunk index per (khalf) -- same for every btile
    last_for_half = {}
    for c, (_, _, kh) in enumerate(meta):
        last_for_half[kh] = c

    with TileContext(nc) as tc:
        with (
            tc.tile_pool(name="const", bufs=1) as constp,
            tc.tile_pool(name="wpool", bufs=4) as wpool,
            tc.tile_pool(name="vec", bufs=4) as vecp,
            tc.tile_pool(name="gpsum", bufs=2, space="PSUM") as gps,
            tc.tile_pool(name="accp", bufs=1, space="PSUM") as accp,
        ):
            # resident data: x/y transposed halves, hi/lo
            def load_pair(dram, nm):
                t0 = constp.tile([P, b_core], f16, name=f"{nm}0", tag=f"{nm}0")
                t1 = constp.tile([P, b_core], f16, name=f"{nm}1", tag=f"{nm}1")
                nc.sync.dma_start(out=t0[:], in_=dram[0:P, :])
                nc.sync.dma_start(out=t1[:], in_=dram[P:2 * P, :])
                return [t0, t1]

            xh = load_pair(xt_hi, "xh")
            xl = load_pair(xt_lo, "xl")
            yh = load_pair(yt_hi, "yh")
            yl = load_pair(yt_lo, "yl")
            cvt = constp.tile([P, n_chunks], f32)
            nc.sync.dma_start(out=cvt[:], in_=cv[:, :])

            LAG = 2  # scatter for chunk c issues after gathers for chunk c+LAG
            for b in range(n_bt):
                bs = slice(b * bt, (b + 1) * bt)
                acc = [accp.tile([P, bt], f32, name="acc0", tag="acc0"),
                       accp.tile([P, bt], f32, name="acc1", tag="acc1")]
                started = [False, False]
                state = {}  # chunk -> (wt, xi, hi, lo)
                for cc in range(n_chunks + LAG):
                    if cc < n_chunks:
                        c = cc
                        ih, jh, kh = meta[c]
                        wt = wpool.tile([P, 3 * P], f16)
                        nc.sync.dma_start(out=wt[:],
                                          in_=w[c * P:(c + 1) * P, :])
                        xi = gps.tile([P, bt], f32, tag="xi", bufs=3)
                        yj = gps.tile([P, bt], f32, tag="yj", bufs=3)
                        nc.tensor.matmul(out=xi[:], lhsT=wt[:, 0:P],
                                         rhs=xh[ih][:, bs], start=True,
                                         stop=False)
                        nc.tensor.matmul(out=xi[:], lhsT=wt[:, 0:P],
                                         rhs=xl[ih][:, bs], start=False,
                                         stop=True)
                        nc.tensor.matmul(out=yj[:], lhsT=wt[:, P:2 * P],
                                         rhs=yh[jh][:, bs], start=True,
                                         stop=False)
                        nc.tensor.matmul(out=yj[:], lhsT=wt[:, P:2 * P],
                                         rhs=yl[jh][:, bs], start=False,
                                         stop=True)

                        yjs = vecp.tile([P, bt], f32, tag="yjs")
                        nc.scalar.copy(out=yjs[:], in_=yj[:])
                        vals = vecp.tile([P, bt], f32, tag="vals")
                        nc.vector.scalar_tensor_tensor(
                            out=vals[:], in0=xi[:], scalar=cvt[:, c:c + 1],
                            in1=yjs[:], op0=mybir.AluOpType.mult,
                            op1=mybir.AluOpType.mult)
                        hi = vecp.tile([P, bt], f16, tag="hi")
                        nc.scalar.copy(out=hi[:], in_=vals[:])
                        lo = vecp.tile([P, bt], f16, tag="lo")
                        nc.vector.tensor_tensor(
                            out=lo[:], in0=vals[:], in1=hi[:],
                            op=mybir.AluOpType.subtract)
                        state[c] = (wt, hi, lo)

                    cs = cc - LAG
                    if cs >= 0:
                        wt, hi, lo = state.pop(cs)
                        kh = meta[cs][2]
                        is_last = (cs == last_for_half[kh])
                        nc.tensor.matmul(out=acc[kh][:],
                                         lhsT=wt[:, 2 * P:3 * P],
                                         rhs=hi[:], start=not started[kh],
                                         stop=False)
                        nc.tensor.matmul(out=acc[kh][:],
                                         lhsT=wt[:, 2 * P:3 * P],
                                         rhs=lo[:], start=False, stop=is_last)
                        started[kh] = True

                for kh in range(2):
                    osb = vecp.tile([P, bt], f32, tag="osb")
                    nc.scalar.copy(out=osb[:], in_=acc[kh][:])
                    nc.sync.dma_start(out=out[kh * P:(kh + 1) * P, bs],
                                      in_=osb[:])
    nc.compile()
    return nc


def _fp16_split(a):
    hi = a.astype(np.float16)
    lo = (a - hi.astype(np.float32)).astype(np.float16)
    return hi, lo


def kernel(x, y, idx_i, idx_j, idx_k, coeff, alpha):
    global LAST_RESULTS
    x = np.asarray(x, dtype=np.float32)
    y = np.asarray(y, dtype=np.float32)
    ii = np.asarray(idx_i).astype(np.int64)
    jj = np.asarray(idx_j).astype(np.int64)
    kk = np.asarray(idx_k).astype(np.int64)
    cc = (np.asarray(coeff).astype(np.float64)
          * np.float64(np.asarray(alpha).reshape(-1)[0])).astype(np.float32)

    B, ALG = x.shape
    assert ALG <= 2 * P
    assert B % NCORES == 0
    b_core = B // NCORES
    bt = min(512, b_core)
    assert b_core % bt == 0
    n_bt = b_core // bt

    # ---- host: bucket + pad triples ----
    bucket = (ii // P) * 4 + (jj // P) * 2 + (kk // P)
    order = np.argsort(bucket, kind="stable")
    I, J, K, C = [], [], [], []
    meta = []
    for bkt in range(8):
        sel = order[bucket[order] == bkt]
        n = len(sel)
        if n == 0:
            continue
        pad = (-n) % P
        ih, jh, kh = bkt // 4, (bkt // 2) % 2, bkt % 2
        I.append(np.concatenate([ii[sel], np.full(pad, ih * P, np.int64)]))
        J.append(np.concatenate([jj[sel], np.full(pad, jh * P, np.int64)]))
        K.append(np.concatenate([kk[sel], np.full(pad, kh * P, np.int64)]))
        C.append(np.concatenate([cc[sel], np.zeros(pad, np.float32)]))
        meta += [(ih, jh, kh)] * ((n + pad) // P)
    I = np.concatenate(I); J = np.concatenate(J)
    K = np.concatenate(K); C = np.concatenate(C)
    n_chunks = len(I) // P
    T = np.arange(len(I))
    chunk = T // P
    tl = T % P

    ihc = np.array([m[0] for m in meta]); jhc = np.array([m[1] for m in meta])
    khc = np.array([m[2] for m in meta])

    w = np.zeros((n_chunks * P, 3 * P), np.float16)
    w[chunk * P + (I - ihc[chunk] * P), tl] = 1.0
    w[chunk * P + (J - jhc[chunk] * P), P + tl] = 1.0
    w[chunk * P + tl, 2 * P + (K - khc[chunk] * P)] = 1.0
    cvarr = np.zeros((P, n_chunks), np.float32)
    cvarr[tl, chunk] = C

    # NOTE: one-hot build above relies on each (row, col) being written at
    # most once for Gi/Gj (true: one i per t) -- duplicates in fancy
    # indexing would be fine anyway since the value is constant 1.
    # For S each t writes one k: also unique per row.

    key = (n_chunks, tuple(meta), b_core, bt, n_bt)
    if key not in _PROG_CACHE:
        _PROG_CACHE[key] = _build_program(n_chunks, meta, b_core, bt, n_bt)
    nc = _PROG_CACHE[key]

    # ---- per-core inputs ----
    in_maps = []
    pad_rows = 2 * P - ALG
    for m in range(NCORES):
        xs = x[m * b_core:(m + 1) * b_core].T
        ys = y[m * b_core:(m + 1) * b_core].T
        xs = np.concatenate([xs, np.zeros((pad_rows, b_core), np.float32)], 0)
        ys = np.concatenate([ys, np.zeros((pad_rows, b_core), np.float32)], 0)
        xhi, xlo = _fp16_split(xs)
        yhi, ylo = _fp16_split(ys)
        in_maps.append({
            "xt_hi": xhi, "xt_lo": xlo, "yt_hi": yhi, "yt_lo": ylo,
            "w": w, "cv": cvarr,
        })

    res = run_bass_kernel_spmd(nc, in_maps, core_ids=list(range(NCORES)))
    LAST_RESULTS = res

    outp = np.empty((B, ALG), np.float32)
    for m in range(NCORES):
        outp[m * b_core:(m + 1) * b_core] = res.results[m]["out"][:ALG].T
    return outp


# revision 12
# speedup vs baseline: 1.1957x; 1.1957x over previous
"""Trainium2 Bass kernel for the sparse Lie-bracket bilinear layer.

  out[b, k] = alpha * sum_{t : idx_k[t]==k} coeff[t] * x[b, idx_i[t]] * y[b, idx_j[t]]

Strategy (data-parallel over batch across 8 NeuronCores, no collectives):
  - Host: sort structure-constant triples into 32 buckets by
    (i_range(64), j_range(64), k_half(128)), pad each bucket to a multiple
    of 128 with zero-coeff triples -> chunks of 128 triples.
  - For each chunk build one-hot matrices (fp16, exact):
      Gi [64 i_local, 128 t]   (placed at PE row strip slot_i*64)
      Gj [64 j_local, 128 t]   (placed at row strip slot_j*64)
      S  [128 t, 128 k_local]  (pure 0/1; coeff applied separately in fp32)
    slot_i != slot_j always (an alternate partition-swapped copy of x is
    used when i_range%2 == j_range%2), so the xi and yj gather matmuls run
    CONCURRENTLY on different 64-row strips of the PE array.
  - xT / yT are split into fp16 hi/lo pairs on the host; the two gather
    matmuls accumulate in PSUM, reconstructing exact fp32 gathered values.
  - DVE computes vals = (xi * c) * yj in one fused scalar_tensor_tensor op
    (c = per-partition fp32 coeff scalars). vals is split into fp16 hi/lo
    and scatter-accumulated into persistent PSUM via two more matmuls.
"""

import numpy as np

import concourse.bass as bass  # noqa: F401
import concourse.mybir as mybir
from concourse import bacc
from concourse.tile import TileContext
from concourse.bass_utils import run_bass_kernel_spmd

NCORES = 8
P = 128
H = 64

_PROG_CACHE = {}

LAST_RESULTS = None  # stash for test.py (exec time / profile)


def _build_program(n_chunks, meta, b_core, bt, n_bt):
    """meta: per chunk (i_range, j_range, k_half, slot_i, use_flip_x)."""
    nc = bacc.Bacc("TRN2", target_bir_lowering=False, debug=False,
                   num_devices=NCORES)
    f16 = mybir.dt.float16
    f32 = mybir.dt.float32

    # x in primary and partition-swapped ("flip") range layouts; y primary.
    dins = {}
    for nm in ("xt_hi", "xt_lo", "xf_hi", "xf_lo", "yt_hi", "yt_lo"):
        dins[nm] = nc.dram_tensor(nm, [2 * P, b_core], f16,
                                  kind="ExternalInput")
    w = nc.dram_tensor("w", [n_chunks * P, 3 * P], f16, kind="ExternalInput")
    cv = nc.dram_tensor("cv", [P, n_chunks], f32, kind="ExternalInput")
    out = nc.dram_tensor("out", [2 * P, b_core], f32, kind="ExternalOutput")

    last_for_half = {}
    for c, m in enumerate(meta):
        last_for_half[m[2]] = c

    with TileContext(nc) as tc:
        with (
            tc.tile_pool(name="const", bufs=1) as constp,
            tc.tile_pool(name="wpool", bufs=4) as wpool,
            tc.tile_pool(name="vec", bufs=4) as vecp,
            tc.tile_pool(name="gpsum", bufs=2, space="PSUM") as gps,
            tc.tile_pool(name="accp", bufs=1, space="PSUM") as accp,
        ):
            sb = {}
            for nm in ("xt_hi", "xt_lo", "xf_hi", "xf_lo", "yt_hi", "yt_lo"):
                for hf in range(2):
                    t = constp.tile([P, b_core], f16, name=f"{nm}{hf}",
                                    tag=f"{nm}{hf}")
                    nc.sync.dma_start(out=t[:],
                                      in_=dins[nm][hf * P:(hf + 1) * P, :])
                    sb[(nm, hf)] = t
            cvt = constp.tile([P, n_chunks], f32)
            nc.sync.dma_start(out=cvt[:], in_=cv[:, :])

            def x_src(part, i_range, slot_i, use_flip):
                nm = ("xf_" if use_flip else "xt_") + part
                tile = sb[(nm, i_range // 2)]
                return tile[slot_i * H:(slot_i + 1) * H, :]

            def y_src(part, j_range):
                tile = sb[("yt_" + part, j_range // 2)]
                sj = j_range % 2
                return tile[sj * H:(sj + 1) * H, :]

            LAG = 2
            for b in range(n_bt):
                bs = slice(b * bt, (b + 1) * bt)
                acc = [accp.tile([P, bt], f32, name="acc0", tag="acc0"),
                       accp.tile([P, bt], f32, name="acc1", tag="acc1")]
                started = [False, False]
                state = {}
                for cc in range(n_chunks + LAG):
                    if cc < n_chunks:
                        c = cc
                        ir, jr, kh, si, ufx = meta[c]
                        sj = jr % 2
                        wt = wpool.tile([P, 3 * P], f16)
                        nc.sync.dma_start(out=wt[:],
                                          in_=w[c * P:(c + 1) * P, :])
                        gi = wt[si * H:(si + 1) * H, 0:P]
                        gj = wt[sj * H:(sj + 1) * H, P:2 * P]

                        xi = gps.tile([P, bt], f32, tag="xi", bufs=3)
                        yj = gps.tile([P, bt], f32, tag="yj", bufs=3)
                        # hi pass: xi and yj matmuls occupy different
                        # 64-row strips -> run concurrently on the PE.
                        nc.tensor.matmul(out=xi[:], lhsT=gi,
                                         rhs=x_src("hi", ir, si, ufx)[:, bs],
                                         start=True, stop=False)
                        nc.tensor.matmul(out=yj[:], lhsT=gj,
                                         rhs=y_src("hi", jr)[:, bs],
                                         start=True, stop=False)
                        # lo pass
                        nc.tensor.matmul(out=xi[:], lhsT=gi,
                                         rhs=x_src("lo", ir, si, ufx)[:, bs],
                                         start=False, stop=True)
                        nc.tensor.matmul(out=yj[:], lhsT=gj,
                                         rhs=y_src("lo", jr)[:, bs],
                                         start=False, stop=True)

                        yjs = vecp.tile([P, bt], f32, tag="yjs")
                        nc.scalar.copy(out=yjs[:], in_=yj[:])
                        vals = vecp.tile([P, bt], f32, tag="vals")
                        nc.vector.scalar_tensor_tensor(
                            out=vals[:], in0=xi[:], scalar=cvt[:, c:c + 1],
                            in1=yjs[:], op0=mybir.AluOpType.mult,
                            op1=mybir.AluOpType.mult)
                        hi = vecp.tile([P, bt], f16, tag="hi")
                        nc.scalar.copy(out=hi[:], in_=vals[:])
                        lo = vecp.tile([P, bt], f16, tag="lo")
                        nc.vector.tensor_tensor(
                            out=lo[:], in0=vals[:], in1=hi[:],
                            op=mybir.AluOpType.subtract)
                        state[c] = (wt, hi, lo)

                    cs = cc - LAG
                    if cs >= 0:
                        wt, hi, lo = state.pop(cs)
                        kh = meta[cs][2]
                        is_last = (cs == last_for_half[kh])
                        nc.tensor.matmul(out=acc[kh][:],
                                         lhsT=wt[:, 2 * P:3 * P],
                                         rhs=hi[:], start=not started[kh],
                                         stop=False)
                        nc.tensor.matmul(out=acc[kh][:],
                                         lhsT=wt[:, 2 * P:3 * P],
                                         rhs=lo[:], start=False, stop=is_last)
                        started[kh] = True

                for kh in range(2):
                    osb = vecp.tile([P, bt], f32, tag="osb")
                    nc.scalar.copy(out=osb[:], in_=acc[kh][:])
                    nc.sync.dma_start(out=out[kh * P:(kh + 1) * P, bs],
                                      in_=osb[:])
    nc.compile()
    return nc


def _fp16_split(a):
    hi = a.astype(np.float16)
    lo = (a - hi.astype(np.float32)).astype(np.float16)
    return hi, lo


def _flip_ranges(a):
    """Swap the two 64-row ranges inside each 128-row half."""
    return np.concatenate([a[H:2 * H], a[0:H], a[3 * H:4 * H], a[2 * H:3 * H]])


def kernel(x, y, idx_i, idx_j, idx_k, coeff, alpha):
    global LAST_RESULTS
    x = np.asarray(x, dtype=np.float32)
    y = np.asarray(y, dtype=np.float32)
    ii = np.asarray(idx_i).astype(np.int64)
    jj = np.asarray(idx_j).astype(np.int64)
    kk = np.asarray(idx_k).astype(np.int64)
    cc = (np.asarray(coeff).astype(np.float64)
          * np.float64(np.asarray(alpha).reshape(-1)[0])).astype(np.float32)

    B, ALG = x.shape
    assert ALG <= 2 * P
    assert B % NCORES == 0
    b_core = B // NCORES
    bt = min(512, b_core)
    assert b_core % bt == 0
    n_bt = b_core // bt

    # ---- host: bucket + pad triples ----
    bucket = (ii // H) * 8 + (jj // H) * 2 + (kk // P)
    order = np.argsort(bucket, kind="stable")
    I, J, K, C = [], [], [], []
    meta = []
    for bkt in range(32):
        sel = order[bucket[order] == bkt]
        n = len(sel)
        if n == 0:
            continue
        ir, jr, kh = bkt // 8, (bkt // 2) % 4, bkt % 2
        pad = (-n) % P
        I.append(np.concatenate([ii[sel], np.full(pad, ir * H, np.int64)]))
        J.append(np.concatenate([jj[sel], np.full(pad, jr * H, np.int64)]))
        K.append(np.concatenate([kk[sel], np.full(pad, kh * P, np.int64)]))
        C.append(np.concatenate([cc[sel], np.zeros(pad, np.float32)]))
        # choose PE row strips: yj always at j_range%2; xi must differ.
        sj = jr % 2
        use_flip = (ir % 2 == sj)
        si = 1 - sj if use_flip else ir % 2
        meta += [(ir, jr, kh, si, use_flip)] * ((n + pad) // P)
    I = np.concatenate(I); J = np.concatenate(J)
    K = np.concatenate(K); C = np.concatenate(C)
    n_chunks = len(I) // P
    T = np.arange(len(I))
    chunk = T // P
    tl = T % P

    irc = np.array([m[0] for m in meta]); jrc = np.array([m[1] for m in meta])
    khc = np.array([m[2] for m in meta]); sic = np.array([m[3] for m in meta])
    sjc = jrc % 2

    w = np.zeros((n_chunks * P, 3 * P), np.float16)
    w[chunk * P + sic[chunk] * H + (I - irc[chunk] * H), tl] = 1.0
    w[chunk * P + sjc[chunk] * H + (J - jrc[chunk] * H), P + tl] = 1.0
    w[chunk * P + tl, 2 * P + (K - khc[chunk] * P)] = 1.0
    cvarr = np.zeros((P, n_chunks), np.float32)
    cvarr[tl, chunk] = C

    key = (n_chunks, tuple(meta), b_core, bt, n_bt)
    if key not in _PROG_CACHE:
        _PROG_CACHE[key] = _build_program(n_chunks, meta, b_core, bt, n_bt)
    nc = _PROG_CACHE[key]

    # ---- per-core inputs ----
    in_maps = []
    pad_rows = 2 * P - ALG
    for m in range(NCORES):
        xs = x[m * b_core:(m + 1) * b_core].T
        ys = y[m * b_core:(m + 1) * b_core].T
        xs = np.concatenate([xs, np.zeros((pad_rows, b_core), np.float32)], 0)
        ys = np.concatenate([ys, np.zeros((pad_rows, b_core), np.float32)], 0)
        xhi, xlo = _fp16_split(xs)
        yhi, ylo = _fp16_split(ys)
        in_maps.append({
            "xt_hi": xhi, "xt_lo": xlo,
            "xf_hi": _flip_ranges(xhi), "xf_lo": _flip_ranges(xlo),
            "yt_hi": yhi, "yt_lo": ylo,
            "w": w, "cv": cvarr,
        })

    res = run_bass_kernel_spmd(nc, in_maps, core_ids=list(range(NCORES)))
    LAST_RESULTS = res

    outp = np.empty((B, ALG), np.float32)
    for m in range(NCORES):
        outp[m * b_core:(m + 1) * b_core] = res.results[m]["out"][:ALG].T
    return outp


# revision 13
# speedup vs baseline: 1.6388x; 1.3705x over previous
"""Trainium2 Bass kernel for the sparse Lie-bracket bilinear layer.

  out[b, k] = alpha * sum_{t : idx_k[t]==k} coeff[t] * x[b, idx_i[t]] * y[b, idx_j[t]]

Strategy (data-parallel over batch across 8 NeuronCores, no collectives):
  - Host: sort structure-constant triples into 32 buckets by
    (i_range(64), j_range(64), k_half(128)), pad each bucket to a multiple
    of 128 with zero-coeff triples -> chunks of 128 triples.
  - For each chunk build one-hot matrices (fp16, exact):
      Gi [64 i_local, 128 t]   (placed at PE row strip slot_i*64)
      Gj [64 j_local, 128 t]   (placed at row strip slot_j*64)
      S  [128 t, 128 k_local]  (pure 0/1; coeff applied separately in fp32)
    slot_i != slot_j always (an alternate partition-swapped copy of x is
    used when i_range%2 == j_range%2), so the xi and yj gather matmuls run
    CONCURRENTLY on different 64-row strips of the PE array.
  - xT / yT are split into fp16 hi/lo pairs on the host; the two gather
    matmuls accumulate in PSUM, reconstructing exact fp32 gathered values.
  - DVE computes vals = (xi * c) * yj in one fused scalar_tensor_tensor op
    (c = per-partition fp32 coeff scalars). vals is split into fp16 hi/lo
    and scatter-accumulated into persistent PSUM via two more matmuls.
"""

import numpy as np

import concourse.bass as bass  # noqa: F401
import concourse.mybir as mybir
from concourse import bacc
from concourse.tile import TileContext
from concourse.bass_utils import run_bass_kernel_spmd

NCORES = 8
P = 128
H = 64

_PROG_CACHE = {}

LAST_RESULTS = None  # stash for test.py (exec time / profile)


def _build_program(n_chunks, meta, b_core, bt, n_bt):
    """meta: per chunk (i_range, j_range, k_half, slot_i, use_flip_x)."""
    nc = bacc.Bacc("TRN2", target_bir_lowering=False, debug=False,
                   num_devices=NCORES)
    f16 = mybir.dt.float16
    f32 = mybir.dt.float32

    # x in primary and partition-swapped ("flip") range layouts; y primary.
    dins = {}
    for nm in ("xt_hi", "xt_lo", "xf_hi", "xf_lo", "yt_hi", "yt_lo"):
        dins[nm] = nc.dram_tensor(nm, [2 * P, b_core], f16,
                                  kind="ExternalInput")
    w = nc.dram_tensor("w", [n_chunks * P, 3 * P], f16, kind="ExternalInput")
    cv = nc.dram_tensor("cv", [P, n_chunks], f32, kind="ExternalInput")
    out = nc.dram_tensor("out", [2 * P, b_core], f32, kind="ExternalOutput")

    last_for_half = {}
    for c, m in enumerate(meta):
        last_for_half[m[2]] = c

    with TileContext(nc) as tc:
        with (
            tc.tile_pool(name="const", bufs=1) as constp,
            tc.tile_pool(name="wpool", bufs=4) as wpool,
            tc.tile_pool(name="vec", bufs=4) as vecp,
            tc.tile_pool(name="gpsum", bufs=2, space="PSUM") as gps,
            tc.tile_pool(name="accp", bufs=1, space="PSUM") as accp,
        ):
            sb = {}
            for nm in ("xt_hi", "xt_lo", "xf_hi", "xf_lo", "yt_hi", "yt_lo"):
                for hf in range(2):
                    t = constp.tile([P, b_core], f16, name=f"{nm}{hf}",
                                    tag=f"{nm}{hf}")
                    nc.sync.dma_start(out=t[:],
                                      in_=dins[nm][hf * P:(hf + 1) * P, :])
                    sb[(nm, hf)] = t
            cvt = constp.tile([P, n_chunks], f32)
            nc.sync.dma_start(out=cvt[:], in_=cv[:, :])

            def x_src(part, i_range, slot_i, use_flip):
                nm = ("xf_" if use_flip else "xt_") + part
                tile = sb[(nm, i_range // 2)]
                return tile[slot_i * H:(slot_i + 1) * H, :]

            def y_src(part, j_range):
                tile = sb[("yt_" + part, j_range // 2)]
                sj = j_range % 2
                return tile[sj * H:(sj + 1) * H, :]

            # Per-stage software pipeline. Stage lags (in chunks):
            #   0: DMA w, gathers (PE), yjs copy (ACT)
            #   1: vals = (xi*c)*yjs (DVE), hi = fp16(vals) (ACT)
            #   2: lo = vals - hi (DVE)
            #   3: scatter matmuls (PE)
            # This keeps each engine's FIFO free of same-chunk chains:
            # DVE never waits (its inputs are one stage old), ACT waits
            # only on PE output, PE's scatter inputs are 1-2 stages old.
            SCAT = 3
            for b in range(n_bt):
                bs = slice(b * bt, (b + 1) * bt)
                acc = [accp.tile([P, bt], f32, name="acc0", tag="acc0"),
                       accp.tile([P, bt], f32, name="acc1", tag="acc1")]
                started = [False, False]
                st = {}
                for cc in range(n_chunks + SCAT):
                    if cc < n_chunks:
                        c = cc
                        ir, jr, kh, si, ufx = meta[c]
                        sj = jr % 2
                        wt = wpool.tile([P, 3 * P], f16, bufs=6)
                        nc.sync.dma_start(out=wt[:],
                                          in_=w[c * P:(c + 1) * P, :])
                        gi = wt[si * H:(si + 1) * H, 0:P]
                        gj = wt[sj * H:(sj + 1) * H, P:2 * P]

                        xi = gps.tile([P, bt], f32, tag="xi", bufs=3)
                        yj = gps.tile([P, bt], f32, tag="yj", bufs=3)
                        # hi pass: xi and yj matmuls occupy different
                        # 64-row strips -> run concurrently on the PE.
                        nc.tensor.matmul(out=xi[:], lhsT=gi,
                                         rhs=x_src("hi", ir, si, ufx)[:, bs],
                                         start=True, stop=False)
                        nc.tensor.matmul(out=yj[:], lhsT=gj,
                                         rhs=y_src("hi", jr)[:, bs],
                                         start=True, stop=False)
                        # lo pass
                        nc.tensor.matmul(out=xi[:], lhsT=gi,
                                         rhs=x_src("lo", ir, si, ufx)[:, bs],
                                         start=False, stop=True)
                        nc.tensor.matmul(out=yj[:], lhsT=gj,
                                         rhs=y_src("lo", jr)[:, bs],
                                         start=False, stop=True)

                        yjs = vecp.tile([P, bt], f32, tag="yjs")
                        nc.scalar.copy(out=yjs[:], in_=yj[:])
                        st[c] = {"wt": wt, "xi": xi, "yjs": yjs}

                    c1 = cc - 1
                    if 0 <= c1 < n_chunks:
                        s = st[c1]
                        vals = vecp.tile([P, bt], f32, tag="vals")
                        nc.vector.scalar_tensor_tensor(
                            out=vals[:], in0=s["xi"][:],
                            scalar=cvt[:, c1:c1 + 1], in1=s["yjs"][:],
                            op0=mybir.AluOpType.mult,
                            op1=mybir.AluOpType.mult)
                        hi = vecp.tile([P, bt], f16, tag="hi", bufs=5)
                        nc.scalar.copy(out=hi[:], in_=vals[:])
                        s["vals"] = vals
                        s["hi"] = hi

                    c2 = cc - 2
                    if 0 <= c2 < n_chunks:
                        s = st[c2]
                        lo = vecp.tile([P, bt], f16, tag="lo")
                        nc.vector.tensor_tensor(
                            out=lo[:], in0=s["vals"][:], in1=s["hi"][:],
                            op=mybir.AluOpType.subtract)
                        s["lo"] = lo

                    cs = cc - SCAT
                    if cs >= 0:
                        s = st.pop(cs)
                        kh = meta[cs][2]
                        is_last = (cs == last_for_half[kh])
                        nc.tensor.matmul(out=acc[kh][:],
                                         lhsT=s["wt"][:, 2 * P:3 * P],
                                         rhs=s["hi"][:],
                                         start=not started[kh], stop=False)
                        nc.tensor.matmul(out=acc[kh][:],
                                         lhsT=s["wt"][:, 2 * P:3 * P],
                                         rhs=s["lo"][:], start=False,
                                         stop=is_last)
                        started[kh] = True

                for kh in range(2):
                    osb = vecp.tile([P, bt], f32, tag="osb")
                    nc.scalar.copy(out=osb[:], in_=acc[kh][:])
                    nc.sync.dma_start(out=out[kh * P:(kh + 1) * P, bs],
                                      in_=osb[:])
    nc.compile()
    return nc


def _fp16_split(a):
    hi = a.astype(np.float16)
    lo = (a - hi.astype(np.float32)).astype(np.float16)
    return hi, lo


def _flip_ranges(a):
    """Swap the two 64-row ranges inside each 128-row half."""
    return np.concatenate([a[H:2 * H], a[0:H], a[3 * H:4 * H], a[2 * H:3 * H]])


def kernel(x, y, idx_i, idx_j, idx_k, coeff, alpha):
    global LAST_RESULTS
    x = np.asarray(x, dtype=np.float32)
    y = np.asarray(y, dtype=np.float32)
    ii = np.asarray(idx_i).astype(np.int64)
    jj = np.asarray(idx_j).astype(np.int64)
    kk = np.asarray(idx_k).astype(np.int64)
    cc = (np.asarray(coeff).astype(np.float64)
          * np.float64(np.asarray(alpha).reshape(-1)[0])).astype(np.float32)

    B, ALG = x.shape
    assert ALG <= 2 * P
    assert B % NCORES == 0
    b_core = B // NCORES
    bt = min(512, b_core)
    assert b_core % bt == 0
    n_bt = b_core // bt

    # ---- host: bucket + pad triples ----
    bucket = (ii // H) * 8 + (jj // H) * 2 + (kk // P)
    order = np.argsort(bucket, kind="stable")
    I, J, K, C = [], [], [], []
    meta = []
    for bkt in range(32):
        sel = order[bucket[order] == bkt]
        n = len(sel)
        if n == 0:
            continue
        ir, jr, kh = bkt // 8, (bkt // 2) % 4, bkt % 2
        pad = (-n) % P
        I.append(np.concatenate([ii[sel], np.full(pad, ir * H, np.int64)]))
        J.append(np.concatenate([jj[sel], np.full(pad, jr * H, np.int64)]))
        K.append(np.concatenate([kk[sel], np.full(pad, kh * P, np.int64)]))
        C.append(np.concatenate([cc[sel], np.zeros(pad, np.float32)]))
        # choose PE row strips: yj always at j_range%2; xi must differ.
        sj = jr % 2
        use_flip = (ir % 2 == sj)
        si = 1 - sj if use_flip else ir % 2
        meta += [(ir, jr, kh, si, use_flip)] * ((n + pad) // P)
    I = np.concatenate(I); J = np.concatenate(J)
    K = np.concatenate(K); C = np.concatenate(C)
    n_chunks = len(I) // P
    T = np.arange(len(I))
    chunk = T // P
    tl = T % P

    irc = np.array([m[0] for m in meta]); jrc = np.array([m[1] for m in meta])
    khc = np.array([m[2] for m in meta]); sic = np.array([m[3] for m in meta])
    sjc = jrc % 2

    w = np.zeros((n_chunks * P, 3 * P), np.float16)
    w[chunk * P + sic[chunk] * H + (I - irc[chunk] * H), tl] = 1.0
    w[chunk * P + sjc[chunk] * H + (J - jrc[chunk] * H), P + tl] = 1.0
    w[chunk * P + tl, 2 * P + (K - khc[chunk] * P)] = 1.0
    cvarr = np.zeros((P, n_chunks), np.float32)
    cvarr[tl, chunk] = C

    key = (n_chunks, tuple(meta), b_core, bt, n_bt)
    if key not in _PROG_CACHE:
        _PROG_CACHE[key] = _build_program(n_chunks, meta, b_core, bt, n_bt)
    nc = _PROG_CACHE[key]

    # ---- per-core inputs ----
    in_maps = []
    pad_rows = 2 * P - ALG
    for m in range(NCORES):
        xs = x[m * b_core:(m + 1) * b_core].T
        ys = y[m * b_core:(m + 1) * b_core].T
        xs = np.concatenate([xs, np.zeros((pad_rows, b_core), np.float32)], 0)
        ys = np.concatenate([ys, np.zeros((pad_rows, b_core), np.float32)], 0)
        xhi, xlo = _fp16_split(xs)
        yhi, ylo = _fp16_split(ys)
        in_maps.append({
            "xt_hi": xhi, "xt_lo": xlo,
            "xf_hi": _flip_ranges(xhi), "xf_lo": _flip_ranges(xlo),
            "yt_hi": yhi, "yt_lo": ylo,
            "w": w, "cv": cvarr,
        })

    res = run_bass_kernel_spmd(nc, in_maps, core_ids=list(range(NCORES)))
    LAST_RESULTS = res

    outp = np.empty((B, ALG), np.float32)
    for m in range(NCORES):
        outp[m * b_core:(m + 1) * b_core] = res.results[m]["out"][:ALG].T
    return outp
